# revision 31
# baseline (speedup 1.0000x reference)
"""Adaptive-softmax loss kernel for one TRN2 chip (8 NeuronCores).

Strategy (token-parallel, zero collectives):
  - Each core owns 512 tokens (4 blocks of 128) and computes the FULL
    adaptive softmax for them: complete head logits (20002 cols), plus
    tail logits only for tokens routed there. Tokens are permuted
    host-side so each core's blocks 0..t1b-1 hold tail1-routed tokens,
    the next t2b blocks tail2-routed ones (padded with head-only
    fillers); the mean loss is permutation invariant.
  - With full-vocab-per-token on one core, per-token logsumexp needs no
    cross-core reduction: each core emits a per-partition partial loss
    sum [128, 1] and the host adds the 8 partials (the unshard step).
    No collective_compute at all (saves ~20us/collective latency).
  - Matmuls run in fp8 (e4m3) DoubleRow (K=256/instr); weights
    pre-scaled by 16 for fp8 range, undone via the exp activation's
    scale. Head/tail1 weights stream from DRAM in 1024-col superstrips,
    split across the sync and gpsimd DMA queues (a single queue's
    transfers top out near ~100 GB/s); matmuls are emitted k-major so
    the PE consumes each K-half of a strip the moment it lands.
  - Per (block, superstrip): 8 matmuls accumulate into a 2-bank PSUM
    tile; ScalarE exp's it into an SBUF bf16 tile (freeing the PSUM tile
    immediately); VectorE row-sums PAIRS of exp tiles with one fused
    scalar_tensor_tensor+accum (both SBUF read ports active -> half the
    reduce time), odd-width leftovers via plain tensor_reduce. Cheap
    tail matmuls are interleaved between head blocks so neither ScalarE
    nor the PE ever builds a burst backlog.
  - Label logits via the host-folded label-weight matrix (wlab):
    z[t, lab] = x[t] . wlab[t]; a VectorE scalar_tensor_tensor row-dot
    per block. Final ln/mask/sum math is a handful of tiny ops.
"""
import os
import numpy as np
import ml_dtypes

N_CORES = 8
B, S, H = 4, 1024, 1024
N = B * S                      # 4096 tokens
P = 128
HK = H // P                    # 8 hidden k-tiles
CUT0, CUT1, CUT2 = 20000, 40000, 50000
HEAD_DIM = CUT0 + 2            # 20002
TOK = N // N_CORES             # 512 tokens per core
TBC = TOK // P                 # 4 blocks per core
W_SCALE = 16.0                 # fp8 weight pre-scale (undone in exp)
PROJ1, PROJ2 = 256, 64
SS = 1024                      # superstrip width (2 PSUM banks)
HEAD_PAD = 20016               # 16-aligned head width (14 zero cols)
N_PAD_HEAD = HEAD_PAD - HEAD_DIM   # 14, exp(0)=1 each, subtracted
V1 = CUT1 - CUT0               # 20000 (16-aligned, no pad)
V2 = CUT2 - CUT1               # 10000
BF16_NP = ml_dtypes.bfloat16

LAST_EXEC_NS = None
LAST_TRACE = None
_NC_CACHE = {}


def _widths(total, step=SS):
    out = []
    s = 0
    while s < total:
        out.append(min(step, total - s))
        s += step
    return out


HVW = _widths(HEAD_PAD)   # 19x1024 + 560
T1W = _widths(V1)         # 19x1024 + 544
T2W = _widths(V2)         # 9x1024 + 784
NSH, NS1, NS2 = len(HVW), len(T1W), len(T2W)


def _pair_plan(ws):
    """Full-width strips pair up (one fused TTR row-sum per pair); the
    rest reduce singly. Returns (pair_limit, n_sep_cols)."""
    nfull = sum(1 for w in ws if w == SS)
    pl = nfull - (nfull % 2)
    return pl, pl // 2 + (len(ws) - pl)


PLH, NCH = _pair_plan(HVW)    # 18, 11
PL1, NC1 = _pair_plan(T1W)    # 18, 11
PL2, NC2 = _pair_plan(T2W)    # 8, 6


def _ensure_trace_hook():
    """The image's antenv package lacks axon_hooks; synthesize it and
    register the ctypes NTFF profile hook so trace=True works."""
    import sys
    import types
    try:
        from antenv.axon_hooks import get_axon_ntff_profile_hook  # noqa: F401
        return
    except ImportError:
        pass
    mod = types.ModuleType("antenv.axon_hooks")
    mod._hook = None

    def set_axon_ntff_profile_hook(h):
        mod._hook = h

    def get_axon_ntff_profile_hook():
        return mod._hook

    mod.set_axon_ntff_profile_hook = set_axon_ntff_profile_hook
    mod.get_axon_ntff_profile_hook = get_axon_ntff_profile_hook
    import antenv
    antenv.axon_hooks = mod
    sys.modules["antenv.axon_hooks"] = mod
    try:
        from trn_agent_boot.trn_boot import _ntff_profile_via_ctypes
        hook = _ntff_profile_via_ctypes("/opt/axon/libaxon_pjrt.so")
        if hook is not None:
            mod._hook = hook
    except Exception:
        pass


def _build_graph(cfg):
    t1b, t2b, with_bias = cfg
    z1_tok = t1b * P
    z2_tok = t2b * P

    import concourse.bacc as bacc
    import concourse.mybir as mybir
    import concourse.tile as tile

    BF16 = mybir.dt.bfloat16
    FP8 = mybir.dt.float8e4
    F32 = mybir.dt.float32
    Exp = mybir.ActivationFunctionType.Exp
    Ln = mybir.ActivationFunctionType.Ln
    MUL = mybir.AluOpType.mult
    ADD = mybir.AluOpType.add
    AX = mybir.AxisListType.X
    DR = mybir.MatmulPerfMode.DoubleRow
    K2N = HK // 2                  # 4 fp8 DoubleRow pair k-tiles

    nc = bacc.Bacc("TRN2", target_bir_lowering=False, debug=False,
                   num_devices=N_CORES)

    # fp8 operands use the DoubleRow pair layout [128, HK, F] where
    # [p, 2*k2 + i, f] = X[(2*k2 + i)*128 + p, f]
    xT_d = nc.dram_tensor("xT", [TBC, P, HK, P], FP8, kind="ExternalInput")
    xb_d = nc.dram_tensor("xb", [TBC, P, H], BF16, kind="ExternalInput")
    wb_d = nc.dram_tensor("wb", [TBC, P, H], BF16, kind="ExternalInput")
    hw_d = nc.dram_tensor("hw", [NSH, P, HK, SS], FP8, kind="ExternalInput")
    ow1_d = nc.dram_tensor("ow1", [NS1, P, 2, SS], FP8, kind="ExternalInput")
    ow2_d = nc.dram_tensor("ow2", [NS2, PROJ2, SS], BF16,
                           kind="ExternalInput")
    pw1_d = nc.dram_tensor("pw1", [P, HK, PROJ1], FP8, kind="ExternalInput")
    pw2_d = nc.dram_tensor("pw2", [P, HK, PROJ2], FP8, kind="ExternalInput")
    padm_d = nc.dram_tensor("padm", [P, TBC], F32, kind="ExternalInput")
    m1_d = nc.dram_tensor("m1m", [P, TBC], F32, kind="ExternalInput")
    m2_d = nc.dram_tensor("m2m", [P, TBC], F32, kind="ExternalInput")
    if with_bias:
        hb_d = nc.dram_tensor("hb", [NSH, 1, SS], BF16, kind="ExternalInput")
        ob1_d = nc.dram_tensor("ob1", [NS1, 1, SS], BF16,
                               kind="ExternalInput")
        ob2_d = nc.dram_tensor("ob2", [NS2, 1, SS], BF16,
                               kind="ExternalInput")
        llb_d = nc.dram_tensor("llb", [P, TBC], F32, kind="ExternalInput")
    out_d = nc.dram_tensor("out", [P, 1], F32, kind="ExternalOutput")

    with tile.TileContext(nc) as tc:
        with (
            tc.tile_pool(name="wp", bufs=1) as wp,
            tc.tile_pool(name="hwp", bufs=3) as hwp,
            tc.tile_pool(name="o1p", bufs=3) as o1p,
            tc.tile_pool(name="o2p", bufs=3) as o2p,
            tc.tile_pool(name="xw", bufs=2) as xw,
            tc.tile_pool(name="scr", bufs=16) as scr,
            tc.tile_pool(name="zs", bufs=4, space="PSUM") as zs,
        ):
            # ---- persistent tiles; DMAs ordered by first consumption ----
            # xt in block slices and hw strips in halves so the first
            # matmuls start ~1us in rather than after the full loads
            xt = wp.tile([P, TBC, HK, P], FP8, name="xt", tag="xt")
            nc.sync.dma_start(xt[:, 0], xT_d[0])
            hwt0 = hwp.tile([P, HK, SS], FP8, name="hwt", tag="hwt")
            nc.sync.dma_start(hwt0[:, 0:2, :], hw_d[0, :, 0:2, :])
            nc.gpsimd.dma_start(hwt0[:, 2:4, :], hw_d[0, :, 2:4, :])
            nc.sync.dma_start(hwt0[:, 4:6, :], hw_d[0, :, 4:6, :])
            nc.gpsimd.dma_start(hwt0[:, 6:8, :], hw_d[0, :, 6:8, :])
            for b in range(1, TBC):
                nc.sync.dma_start(xt[:, b], xT_d[b])
            pw1_t = wp.tile([P, HK, PROJ1], FP8, name="pw1_t", tag="pw1")
            nc.sync.dma_start(pw1_t[:], pw1_d[:])
            pw2_t = wp.tile([P, HK, PROJ2], FP8, name="pw2_t", tag="pw2")
            nc.sync.dma_start(pw2_t[:], pw2_d[:])

            # preload the Ln activation table now (ScalarE is idle during
            # the first DMAs) so it doesn't load inside the final phase
            lnw = wp.tile([P, 1], F32, name="lnw", tag="lnw")
            nc.gpsimd.memset(lnw[:], 1.0)
            nc.scalar.activation(lnw[:], lnw[:], Ln)

            p1T = wp.tile([P, 2, z1_tok], FP8, name="p1T", tag="p1T")
            p2T = wp.tile([PROJ2, z2_tok], BF16, name="p2T", tag="p2T")
            sep_h = wp.tile([P, TBC * NCH], F32, name="sep_h", tag="seph")
            sep_1 = wp.tile([P, t1b * NC1], F32, name="sep_1", tag="sep1")
            sep_2 = wp.tile([P, t2b * NC2], F32, name="sep_2", tag="sep2")
            ll_loc = wp.tile([P, TBC], F32, name="ll_loc", tag="llloc")
            if with_bias:
                ones_bf = wp.tile([1, P], BF16, name="ones_bf", tag="onesb")
                nc.gpsimd.memset(ones_bf[:], 1.0)

            def z_chunks(zt, w, mm_emit, bias_tile=None):
                """Fill zt[:, 0:w] by 512-col chunks via mm_emit(sl, first)."""
                for c0 in range(0, w, 512):
                    cw = min(512, w - c0)
                    sl = slice(c0, c0 + cw)
                    first = True
                    if bias_tile is not None:
                        nc.tensor.matmul(zt[0:P, sl], ones_bf[:],
                                         bias_tile[:, sl],
                                         start=True, stop=False)
                        first = False
                    mm_emit(sl, first)

            # exp goes PSUM -> SBUF bf16 (frees the PSUM tile at the ACT,
            # keeping the PE fed); row-sums run on VectorE, fusing PAIRS of
            # full-width exp tiles into one scalar_tensor_tensor with
            # accum_out (both SBUF read ports active -> half the reduce
            # time; accum = sum(A*1 + B))
            pending = {}
            next_col = {}
            dump = wp.tile([P, SS], BF16, name="dump", tag="dump")

            def exp_red(key, s, pair_limit, zt, w, sep, ncols, scale):
                et = scr.tile([P, SS], BF16, name="et", tag="et")
                nc.scalar.activation(et[:, 0:w], zt[:, 0:w], Exp,
                                     scale=scale)
                col = next_col.get(key, 0)
                base = key[1] * ncols
                if s < pair_limit:
                    if key in pending:
                        pt = pending.pop(key)
                        nc.vector.scalar_tensor_tensor(
                            out=dump[:, 0:w], in0=pt[:, 0:w], scalar=1.0,
                            in1=et[:, 0:w], op0=MUL, op1=ADD,
                            accum_out=sep[:, base + col:base + col + 1])
                        next_col[key] = col + 1
                    else:
                        pending[key] = et
                else:
                    nc.vector.tensor_reduce(
                        out=sep[:, base + col:base + col + 1],
                        in_=et[:, 0:w], axis=AX, op=ADD)
                    next_col[key] = col + 1

            # ---- main rounds: per superstrip r, head (4 blocks) + tail
            # tiles, interleaved so cheap tail matmuls sit between the
            # expensive head ones and ScalarE never builds a burst backlog
            for r in range(NSH):
                if r == 0:
                    hwt = hwt0
                else:
                    hwt = hwp.tile([P, HK, SS], FP8, name="hwt", tag="hwt")
                    nc.sync.dma_start(hwt[:, 0:HK // 2, :],
                                      hw_d[r, :, 0:HK // 2, :])
                    nc.gpsimd.dma_start(hwt[:, HK // 2:HK, :],
                                        hw_d[r, :, HK // 2:HK, :])
                if with_bias:
                    hbt = hwp.tile([1, SS], BF16, name="hbt", tag="hbt")
                    nc.sync.dma_start(hbt[:], hb_d[r])

                tail_jobs = []
                if r < NS1:
                    o1t = o1p.tile([P, 2, SS], FP8, name="o1t", tag="o1t")
                    nc.gpsimd.dma_start(o1t[:], ow1_d[r])
                    o1bt = None
                    if with_bias:
                        o1bt = o1p.tile([1, SS], BF16, name="o1bt",
                                        tag="o1bt")
                        nc.sync.dma_start(o1bt[:], ob1_d[r])
                    w1 = T1W[r]

                    def t1_job(b, o1t=o1t, o1bt=o1bt, w1=w1, r=r):
                        zt = zs.tile([P, SS], F32, name="zt1", tag="zs")

                        def t1_mm(sl, first):
                            nc.tensor.matmul(
                                zt[0:P, sl],
                                p1T[:, :, b * P:(b + 1) * P],
                                o1t[:, :, sl],
                                start=first, stop=True, perf_mode=DR)

                        z_chunks(zt, w1, t1_mm, o1bt)
                        exp_red(("1", b), r, PL1, zt, w1, sep_1, NC1,
                                1.0 / W_SCALE)

                    for b in range(t1b):
                        tail_jobs.append((t1_job, b))
                if r % 2 == 0 and r // 2 < NS2:
                    s2 = r // 2
                    o2t = o2p.tile([PROJ2, SS], BF16, name="o2t", tag="o2t")
                    nc.gpsimd.dma_start(o2t[:], ow2_d[s2])
                    o2bt = None
                    if with_bias:
                        o2bt = o2p.tile([1, SS], BF16, name="o2bt",
                                        tag="o2bt")
                        nc.gpsimd.dma_start(o2bt[:], ob2_d[s2])
                    w2 = T2W[s2]

                    def t2_job(b, o2t=o2t, o2bt=o2bt, w2=w2, s2=s2):
                        zt = zs.tile([P, SS], F32, name="zt2", tag="zs")

                        def t2_mm(sl, first):
                            nc.tensor.matmul(
                                zt[0:P, sl],
                                p2T[:, b * P:(b + 1) * P],
                                o2t[:, sl],
                                start=first, stop=True)

                        z_chunks(zt, w2, t2_mm, o2bt)
                        exp_red(("2", b), s2, PL2, zt, w2, sep_2, NC2, 1.0)

                    for b in range(t2b):
                        tail_jobs.append((t2_job, b))

                w = HVW[r]
                for b in range(TBC):
                    zt = zs.tile([P, SS], F32, name="zt", tag="zs")

                    # k-major emission: all 512-chunks at k2 before moving to
                    # the next k2, so the PE streams each K-half of the
                    # weight strip as soon as its DMA lands
                    chunks = []
                    for c0 in range(0, w, 512):
                        cw = min(512, w - c0)
                        sl = slice(c0, c0 + cw)
                        if with_bias:
                            nc.tensor.matmul(zt[0:P, sl], ones_bf[:],
                                             hbt[:, sl],
                                             start=True, stop=False)
                        chunks.append(sl)
                    for k2 in range(K2N):
                        for sl in chunks:
                            nc.tensor.matmul(
                                zt[0:P, sl],
                                xt[:, b, 2 * k2:2 * k2 + 2, :],
                                hwt[:, 2 * k2:2 * k2 + 2, sl],
                                start=(k2 == 0 and not with_bias),
                                stop=(k2 == K2N - 1), perf_mode=DR)
                    exp_red(("h", b), r, PLH, zt, w, sep_h, NCH,
                            1.0 / W_SCALE)
                    if r > 0 and b < len(tail_jobs):
                        job, jb = tail_jobs[b]
                        job(jb)

                if r == 0:
                    # projections (tiny): tail proj activations for the
                    # routed zones, transposed into DoubleRow layouts
                    for m in range(PROJ1 // P):
                        acc = zs.tile([P, SS], F32, name="acc1", tag="zs")
                        for bb in range(t1b):
                            for k2 in range(K2N):
                                nc.tensor.matmul(
                                    acc[0:P, bb * P:(bb + 1) * P],
                                    pw1_t[:, 2 * k2:2 * k2 + 2,
                                          m * P:(m + 1) * P],
                                    xt[:, bb, 2 * k2:2 * k2 + 2, :],
                                    start=(k2 == 0), stop=(k2 == K2N - 1),
                                    perf_mode=DR)
                        nc.vector.tensor_copy(out=p1T[:, m, :],
                                              in_=acc[:, 0:z1_tok])
                    acc2 = zs.tile([P, SS], F32, name="acc2", tag="zs")
                    for bb in range(t2b):
                        for k2 in range(K2N):
                            nc.tensor.matmul(
                                acc2[0:PROJ2, bb * P:(bb + 1) * P],
                                pw2_t[:, 2 * k2:2 * k2 + 2, 0:PROJ2],
                                xt[:, t1b + bb, 2 * k2:2 * k2 + 2, :],
                                start=(k2 == 0), stop=(k2 == K2N - 1),
                                perf_mode=DR)
                    nc.vector.tensor_copy(out=p2T[:],
                                          in_=acc2[0:PROJ2, 0:z2_tok])
                    # round 0's tail tiles run after the projections
                    for job, jb in tail_jobs:
                        job(jb)

                # label-logit row-dots, one block per early round; DMAs on
                # the idle gpsimd queue so the sync FIFO never stalls the
                # weight stream
                if 3 <= r < 3 + 2 * TBC:
                    bl, half = divmod(r - 3, 2)
                    if half == 0:
                        xe = xw.tile([P, H], BF16, name="xe", tag="xe")
                        nc.sync.dma_start(xe[:], xb_d[bl])
                        xw_pend = xe
                    else:
                        we = xw.tile([P, H], BF16, name="we", tag="we")
                        nc.sync.dma_start(we[:], wb_d[bl])
                        lsc = xw.tile([P, H], BF16, name="lsc", tag="lsc")
                        nc.vector.scalar_tensor_tensor(
                            out=lsc[:], in0=xw_pend[:], scalar=1.0,
                            in1=we[:], op0=MUL, op1=MUL,
                            accum_out=ll_loc[:, bl:bl + 1])

                if r == 2:
                    padm_t = wp.tile([P, TBC], F32, name="padm_t", tag="padm")
                    nc.gpsimd.dma_start(padm_t[:], padm_d[:])
                    m1_t = wp.tile([P, TBC], F32, name="m1_t", tag="m1")
                    nc.gpsimd.dma_start(m1_t[:], m1_d[:])
                    m2_t = wp.tile([P, TBC], F32, name="m2_t", tag="m2")
                    nc.gpsimd.dma_start(m2_t[:], m2_d[:])
                    if with_bias:
                        llb_t = wp.tile([P, TBC], F32, name="llb_t",
                                        tag="llb")
                        nc.gpsimd.dma_start(llb_t[:], llb_d[:])

            # ---- final: per-token ln, mask, subtract label logit ----
            sev_h = sep_h.rearrange("p (b s) -> p b s", s=NCH)
            sh = wp.tile([P, TBC], F32, name="sh", tag="sh")
            nc.vector.tensor_reduce(out=sh[:], in_=sev_h, axis=AX, op=ADD)
            nc.vector.tensor_scalar_add(sh[:], sh[:], -float(N_PAD_HEAD))
            ln_h = wp.tile([P, TBC], F32, name="ln_h", tag="lnh")
            nc.scalar.activation(ln_h[:], sh[:], Ln)

            sev_1 = sep_1.rearrange("p (b s) -> p b s", s=NC1)
            s1 = wp.tile([P, t1b], F32, name="s1", tag="s1")
            nc.vector.tensor_reduce(out=s1[:], in_=sev_1, axis=AX, op=ADD)
            ln_1 = wp.tile([P, t1b], F32, name="ln_1", tag="ln1")
            nc.scalar.activation(ln_1[:], s1[:], Ln)

            sev_2 = sep_2.rearrange("p (b s) -> p b s", s=NC2)
            s2t = wp.tile([P, t2b], F32, name="s2t", tag="s2t")
            nc.vector.tensor_reduce(out=s2t[:], in_=sev_2, axis=AX, op=ADD)
            ln_2 = wp.tile([P, t2b], F32, name="ln_2", tag="ln2")
            nc.scalar.activation(ln_2[:], s2t[:], Ln)

            acc_l = wp.tile([P, TBC], F32, name="acc_l", tag="accl")
            tmp_l = wp.tile([P, TBC], F32, name="tmp_l", tag="tmpl")
            nc.vector.tensor_mul(out=acc_l[:], in0=padm_t[:], in1=ln_h[:])
            nc.vector.tensor_mul(out=tmp_l[:, 0:t1b], in0=m1_t[:, 0:t1b],
                                 in1=ln_1[:])
            nc.vector.tensor_add(out=acc_l[:, 0:t1b], in0=acc_l[:, 0:t1b],
                                 in1=tmp_l[:, 0:t1b])
            nc.vector.tensor_mul(out=tmp_l[:, 0:t2b],
                                 in0=m2_t[:, t1b:t1b + t2b], in1=ln_2[:])
            nc.vector.tensor_add(out=acc_l[:, t1b:t1b + t2b],
                                 in0=acc_l[:, t1b:t1b + t2b],
                                 in1=tmp_l[:, 0:t2b])
            nc.vector.tensor_sub(out=acc_l[:], in0=acc_l[:], in1=ll_loc[:])
            if with_bias:
                nc.vector.tensor_sub(out=acc_l[:], in0=acc_l[:],
                                     in1=llb_t[:])

            lred = wp.tile([P, 1], F32, name="lred", tag="lred")
            nc.vector.tensor_reduce(out=lred[:], in_=acc_l[:],
                                    axis=AX, op=ADD)
            nc.sync.dma_start(out_d[:], lred[:])

    nc.compile()
    return nc


def _get_nc(cfg):
    if cfg not in _NC_CACHE:
        _NC_CACHE[cfg] = _build_graph(cfg)
    return _NC_CACHE[cfg]


def kernel(inp, labels, head_w, head_b, t1_pw, t1_pb, t1_ow, t1_ob,
           t2_pw, t2_pb, t2_ow, t2_ob):
    global LAST_EXEC_NS, LAST_TRACE
    from concourse.bass_utils import run_bass_kernel_spmd

    inp = np.asarray(inp, dtype=np.float32)
    labels = np.asarray(labels)
    head_w = np.asarray(head_w, dtype=np.float32)
    head_b = np.asarray(head_b, dtype=np.float32)
    t1_pw = np.asarray(t1_pw, dtype=np.float32)
    t1_pb = np.asarray(t1_pb, dtype=np.float32)
    t1_ow = np.asarray(t1_ow, dtype=np.float32)
    t1_ob = np.asarray(t1_ob, dtype=np.float32)
    t2_pw = np.asarray(t2_pw, dtype=np.float32)
    t2_pb = np.asarray(t2_pb, dtype=np.float32)
    t2_ow = np.asarray(t2_ow, dtype=np.float32)
    t2_ob = np.asarray(t2_ob, dtype=np.float32)

    x0 = np.ascontiguousarray(inp.reshape(N, H))
    lab0 = labels.reshape(N).astype(np.int64)

    # token permutation: per core, tail1 zone first, then tail2 zone,
    # head-only fill
    m1_0 = (lab0 >= CUT0) & (lab0 < CUT1)
    m2_0 = lab0 >= CUT1
    idx1 = np.where(m1_0)[0]
    idx2 = np.where(m2_0)[0]
    idx0 = np.where(~(m1_0 | m2_0))[0]
    n1, n2 = len(idx1), len(idx2)
    t1b = max(1, -(-n1 // (N_CORES * P)))   # per-core tail1 blocks
    t2b = max(1, -(-n2 // (N_CORES * P)))
    if t1b + t2b > TBC:
        raise NotImplementedError(
            "label distribution exceeds routed-zone capacity")
    chunks1 = np.array_split(idx1, N_CORES)
    chunks2 = np.array_split(idx2, N_CORES)
    fill = list(idx0)
    fp = 0
    perms = []
    for c in range(N_CORES):
        parts = [chunks1[c]]
        need = t1b * P - len(chunks1[c])
        parts.append(np.asarray(fill[fp:fp + need], dtype=np.int64))
        fp += need
        parts.append(chunks2[c])
        need = t2b * P - len(chunks2[c])
        parts.append(np.asarray(fill[fp:fp + need], dtype=np.int64))
        fp += need
        need = TOK - t1b * P - t2b * P
        parts.append(np.asarray(fill[fp:fp + need], dtype=np.int64))
        fp += need
        perms.append(np.concatenate(parts))
    assert fp == len(fill)
    perm = np.concatenate(perms)
    assert perm.size == N

    x = x0[perm]
    lab = lab0[perm]

    m1 = (lab >= CUT0) & (lab < CUT1)
    m2 = lab >= CUT1
    pad = (lab != 0).astype(np.float32)
    head_labels = np.where(m1, CUT0, np.where(m2, CUT0 + 1, lab))
    lab1 = np.clip(lab - CUT0, 0, V1 - 1)
    lab2 = np.clip(lab - CUT1, 0, V2 - 1)
    m1f = m1.astype(np.float32)
    m2f = m2.astype(np.float32)

    with_bias = any(float(np.abs(b).max()) != 0.0
                    for b in (head_b, t1_pb, t1_ob, t2_pb, t2_ob))

    # effective label-weight columns, tails folded through their projections
    wl = head_w[:, head_labels]                      # [H, N]
    wl1 = t1_pw @ t1_ow[:, lab1]                     # [H, N]
    wl2 = t2_pw @ t2_ow[:, lab2]                     # [H, N]
    WLAB = (wl + m1f[None, :] * wl1 + m2f[None, :] * wl2) * pad[None, :]
    wlab_nat = np.ascontiguousarray(WLAB.T).astype(BF16_NP)      # [N, H]
    x_bf = x.astype(BF16_NP)

    import concourse.mybir as _mybir
    FP8_NP = _mybir.dt.np(_mybir.dt.float8e4)

    def pack_pairs(Xt):
        # [K, F] -> [128, K//128, F] with [p, kk, f] = Xt[kk*128 + p, f]
        K_, F_ = Xt.shape
        return np.ascontiguousarray(
            Xt.reshape(K_ // P, P, F_).transpose(1, 0, 2))

    def pack_strips(Xt, nss):
        # [K, nss*SS] -> [nss, 128, K//128, SS] strip-major pair layout
        K_ = Xt.shape[0]
        return np.ascontiguousarray(
            Xt.reshape(K_ // P, P, nss, SS).transpose(2, 1, 0, 3))

    hw_s = np.zeros((H, NSH * SS), dtype=np.float32)
    hw_s[:, :HEAD_DIM] = head_w * W_SCALE
    hw_pack = pack_strips(hw_s, NSH).astype(FP8_NP)

    o1_s = np.zeros((PROJ1, NS1 * SS), dtype=np.float32)
    o1_s[:, :V1] = t1_ow
    ow1_pack = pack_strips(o1_s, NS1).astype(FP8_NP)

    o2_s = np.zeros((PROJ2, NS2 * SS), dtype=np.float32)
    o2_s[:, :V2] = t2_ow * (1.0 / W_SCALE)
    ow2_pack = np.ascontiguousarray(
        o2_s.reshape(PROJ2, NS2, SS).transpose(1, 0, 2)).astype(BF16_NP)

    pw1_f8 = pack_pairs(t1_pw * W_SCALE).astype(FP8_NP)
    pw2_f8 = pack_pairs(t2_pw * W_SCALE).astype(FP8_NP)

    if with_bias:
        hb_s = np.zeros((NSH * SS,), dtype=np.float32)
        hb_s[:HEAD_DIM] = head_b * W_SCALE
        hb_pack = hb_s.reshape(NSH, 1, SS).astype(BF16_NP)
        ob1_s = np.zeros((NS1 * SS,), dtype=np.float32)
        ob1_s[:V1] = (t1_ob + t1_pb @ t1_ow) * W_SCALE
        ob1_pack = ob1_s.reshape(NS1, 1, SS).astype(BF16_NP)
        ob2_s = np.zeros((NS2 * SS,), dtype=np.float32)
        ob2_s[:V2] = t2_ob + t2_pb @ t2_ow
        ob2_pack = ob2_s.reshape(NS2, 1, SS).astype(BF16_NP)
        llb_vec = pad * (head_b[head_labels]
                         + m1f * (t1_pb @ t1_ow[:, lab1] + t1_ob[lab1])
                         + m2f * (t2_pb @ t2_ow[:, lab2] + t2_ob[lab2]))

    def to_ptb(v):
        return np.ascontiguousarray(
            v.reshape(TBC, P).T).astype(np.float32)   # [P, TBC]

    in_maps = []
    for c in range(N_CORES):
        tsl = slice(c * TOK, (c + 1) * TOK)
        xc = x[tsl]
        xt_blocks = np.stack([
            pack_pairs(np.ascontiguousarray(xc[b * P:(b + 1) * P].T))
            for b in range(TBC)])                    # [TBC, P, HK, 128]
        m = {
            "xT": xt_blocks.astype(FP8_NP),
            "xb": np.ascontiguousarray(x_bf[tsl].reshape(TBC, P, H)),
            "wb": np.ascontiguousarray(wlab_nat[tsl].reshape(TBC, P, H)),
            "hw": hw_pack,
            "ow1": ow1_pack,
            "ow2": ow2_pack,
            "pw1": pw1_f8,
            "pw2": pw2_f8,
            "padm": to_ptb(pad[tsl]),
            "m1m": to_ptb(m1f[tsl]),
            "m2m": to_ptb(m2f[tsl]),
        }
        if with_bias:
            m["hb"] = hb_pack
            m["ob1"] = ob1_pack
            m["ob2"] = ob2_pack
            m["llb"] = to_ptb(llb_vec[tsl])
        in_maps.append(m)

    nc = _get_nc((t1b, t2b, with_bias))
    trace = bool(os.environ.get("KERNEL_TRACE"))
    if trace:
        _ensure_trace_hook()
    # the fleet occasionally throws transient NRT device errors on the first
    # execution after a crashed run; retry a couple of times
    res = None
    for attempt in range(3):
        try:
            res = run_bass_kernel_spmd(
                nc, in_maps, core_ids=list(range(N_CORES)), trace=trace)
            break
        except Exception:
            if attempt == 2:
                raise
            import time
            time.sleep(3.0)
    LAST_EXEC_NS = res.exec_time_ns
    LAST_TRACE = res.instructions_and_trace
    total = 0.0
    for c in range(N_CORES):
        total += float(res.results[c]["out"].astype(np.float64).sum())
    return np.float32(total / N)


# revision 32
# speedup vs baseline: 1.1789x; 1.1789x over previous
"""Adaptive-softmax loss kernel for one TRN2 chip (8 NeuronCores).

Strategy (token-parallel, zero collectives):
  - Each core owns 512 tokens (4 blocks of 128) and computes the FULL
    adaptive softmax for them: complete head logits (20002 cols), plus
    tail logits only for tokens routed there. Tokens are permuted
    host-side so each core's blocks 0..t1b-1 hold tail1-routed tokens,
    the next t2b blocks tail2-routed ones (padded with head-only
    fillers); the mean loss is permutation invariant.
  - With full-vocab-per-token on one core, per-token logsumexp needs no
    cross-core reduction: each core emits a per-partition partial loss
    sum [128, 1] and the host adds the 8 partials (the unshard step).
    No collective_compute at all (saves ~20us/collective latency).
  - Matmuls run in fp8 (e4m3) DoubleRow (K=256/instr); weights
    pre-scaled by 16 for fp8 range, undone via the exp activation's
    scale. Head/tail1 weights stream from DRAM in 1024-col superstrips,
    split across the sync and gpsimd DMA queues (a single queue's
    transfers top out near ~100 GB/s); matmuls are emitted k-major so
    the PE consumes each K-half of a strip the moment it lands.
  - Per (block, superstrip): 8 matmuls accumulate into a 2-bank PSUM
    tile; ScalarE exp's it into an SBUF bf16 tile (freeing the PSUM tile
    immediately); VectorE row-sums PAIRS of exp tiles with one fused
    scalar_tensor_tensor+accum (both SBUF read ports active -> half the
    reduce time), odd-width leftovers via plain tensor_reduce. Cheap
    tail matmuls are interleaved between head blocks so neither ScalarE
    nor the PE ever builds a burst backlog.
  - Label logits via the host-folded label-weight matrix (wlab):
    z[t, lab] = x[t] . wlab[t]; a VectorE scalar_tensor_tensor row-dot
    per block. Final ln/mask/sum math is a handful of tiny ops.
"""
import os
import numpy as np
import ml_dtypes

N_CORES = 8
B, S, H = 4, 1024, 1024
N = B * S                      # 4096 tokens
P = 128
HK = H // P                    # 8 hidden k-tiles
CUT0, CUT1, CUT2 = 20000, 40000, 50000
HEAD_DIM = CUT0 + 2            # 20002
TOK = N // N_CORES             # 512 tokens per core
TBC = TOK // P                 # 4 blocks per core
W_SCALE = 16.0                 # fp8 weight pre-scale (undone in exp)
PROJ1, PROJ2 = 256, 64
SS = 1024                      # superstrip width (2 PSUM banks)
HEAD_PAD = 20016               # 16-aligned head width (14 zero cols)
N_PAD_HEAD = HEAD_PAD - HEAD_DIM   # 14, exp(0)=1 each, subtracted
V1 = CUT1 - CUT0               # 20000 (16-aligned, no pad)
V2 = CUT2 - CUT1               # 10000
BF16_NP = ml_dtypes.bfloat16

LAST_EXEC_NS = None
LAST_TRACE = None
_NC_CACHE = {}


def _widths(total, step=SS):
    out = []
    s = 0
    while s < total:
        out.append(min(step, total - s))
        s += step
    return out


HVW = _widths(HEAD_PAD)   # 19x1024 + 560
T1W = _widths(V1)         # 19x1024 + 544
T2W = _widths(V2)         # 9x1024 + 784
NSH, NS1, NS2 = len(HVW), len(T1W), len(T2W)


def _pair_plan(ws):
    """Full-width strips pair up (one fused TTR row-sum per pair); the
    rest reduce singly. Returns (pair_limit, n_sep_cols)."""
    nfull = sum(1 for w in ws if w == SS)
    pl = nfull - (nfull % 2)
    return pl, pl // 2 + (len(ws) - pl)


PLH, NCH = _pair_plan(HVW)    # 18, 11
PL1, NC1 = _pair_plan(T1W)    # 18, 11
PL2, NC2 = _pair_plan(T2W)    # 8, 6


def _ensure_trace_hook():
    """The image's antenv package lacks axon_hooks; synthesize it and
    register the ctypes NTFF profile hook so trace=True works."""
    import sys
    import types
    try:
        from antenv.axon_hooks import get_axon_ntff_profile_hook  # noqa: F401
        return
    except ImportError:
        pass
    mod = types.ModuleType("antenv.axon_hooks")
    mod._hook = None

    def set_axon_ntff_profile_hook(h):
        mod._hook = h

    def get_axon_ntff_profile_hook():
        return mod._hook

    mod.set_axon_ntff_profile_hook = set_axon_ntff_profile_hook
    mod.get_axon_ntff_profile_hook = get_axon_ntff_profile_hook
    import antenv
    antenv.axon_hooks = mod
    sys.modules["antenv.axon_hooks"] = mod
    try:
        from trn_agent_boot.trn_boot import _ntff_profile_via_ctypes
        hook = _ntff_profile_via_ctypes("/opt/axon/libaxon_pjrt.so")
        if hook is not None:
            mod._hook = hook
    except Exception:
        pass


def _build_graph(cfg):
    t1b, t2b, with_bias = cfg
    z1_tok = t1b * P
    z2_tok = t2b * P

    import concourse.bacc as bacc
    import concourse.mybir as mybir
    import concourse.tile as tile

    BF16 = mybir.dt.bfloat16
    FP8 = mybir.dt.float8e4
    F32 = mybir.dt.float32
    Exp = mybir.ActivationFunctionType.Exp
    Ln = mybir.ActivationFunctionType.Ln
    MUL = mybir.AluOpType.mult
    ADD = mybir.AluOpType.add
    AX = mybir.AxisListType.X
    DR = mybir.MatmulPerfMode.DoubleRow
    K2N = HK // 2                  # 4 fp8 DoubleRow pair k-tiles

    nc = bacc.Bacc("TRN2", target_bir_lowering=False, debug=False,
                   num_devices=N_CORES)

    # fp8 operands use the DoubleRow pair layout [128, HK, F] where
    # [p, 2*k2 + i, f] = X[(2*k2 + i)*128 + p, f]
    xT_d = nc.dram_tensor("xT", [TBC, P, HK, P], FP8, kind="ExternalInput")
    xb_d = nc.dram_tensor("xb", [TBC, P, H], BF16, kind="ExternalInput")
    wb_d = nc.dram_tensor("wb", [TBC, P, H], BF16, kind="ExternalInput")
    hw_d = nc.dram_tensor("hw", [NSH, P, HK, SS], FP8, kind="ExternalInput")
    ow1_d = nc.dram_tensor("ow1", [NS1, P, 2, SS], FP8, kind="ExternalInput")
    ow2_d = nc.dram_tensor("ow2", [NS2, PROJ2, SS], BF16,
                           kind="ExternalInput")
    pw1_d = nc.dram_tensor("pw1", [P, HK, PROJ1], FP8, kind="ExternalInput")
    pw2_d = nc.dram_tensor("pw2", [P, HK, PROJ2], FP8, kind="ExternalInput")
    padm_d = nc.dram_tensor("padm", [P, TBC], F32, kind="ExternalInput")
    m1_d = nc.dram_tensor("m1m", [P, TBC], F32, kind="ExternalInput")
    m2_d = nc.dram_tensor("m2m", [P, TBC], F32, kind="ExternalInput")
    if with_bias:
        hb_d = nc.dram_tensor("hb", [NSH, 1, SS], BF16, kind="ExternalInput")
        ob1_d = nc.dram_tensor("ob1", [NS1, 1, SS], BF16,
                               kind="ExternalInput")
        ob2_d = nc.dram_tensor("ob2", [NS2, 1, SS], BF16,
                               kind="ExternalInput")
        llb_d = nc.dram_tensor("llb", [P, TBC], F32, kind="ExternalInput")
    out_d = nc.dram_tensor("out", [P, 1], F32, kind="ExternalOutput")

    with tile.TileContext(nc) as tc:
        with (
            tc.tile_pool(name="wp", bufs=1) as wp,
            tc.tile_pool(name="hwp", bufs=3) as hwp,
            tc.tile_pool(name="o1p", bufs=3) as o1p,
            tc.tile_pool(name="o2p", bufs=3) as o2p,
            tc.tile_pool(name="xw", bufs=2) as xw,
            tc.tile_pool(name="scr", bufs=16) as scr,
            tc.tile_pool(name="zs", bufs=4, space="PSUM") as zs,
        ):
            # ---- persistent tiles; DMAs ordered by first consumption ----
            # xt in block slices and hw strips in halves so the first
            # matmuls start ~1us in rather than after the full loads
            xt = wp.tile([P, TBC, HK, P], FP8, name="xt", tag="xt")
            nc.sync.dma_start(xt[:, 0], xT_d[0])
            hwt0 = hwp.tile([P, HK, SS], FP8, name="hwt", tag="hwt")
            nc.sync.dma_start(hwt0[:, 0:2, :], hw_d[0, :, 0:2, :])
            nc.gpsimd.dma_start(hwt0[:, 2:4, :], hw_d[0, :, 2:4, :])
            nc.sync.dma_start(hwt0[:, 4:6, :], hw_d[0, :, 4:6, :])
            nc.gpsimd.dma_start(hwt0[:, 6:8, :], hw_d[0, :, 6:8, :])
            for b in range(1, TBC):
                nc.sync.dma_start(xt[:, b], xT_d[b])
            pw1_t = wp.tile([P, HK, PROJ1], FP8, name="pw1_t", tag="pw1")
            nc.sync.dma_start(pw1_t[:], pw1_d[:])
            pw2_t = wp.tile([P, HK, PROJ2], FP8, name="pw2_t", tag="pw2")
            nc.sync.dma_start(pw2_t[:], pw2_d[:])

            # preload the Ln activation table now (ScalarE is idle during
            # the first DMAs) so it doesn't load inside the final phase
            lnw = wp.tile([P, 1], F32, name="lnw", tag="lnw")
            nc.gpsimd.memset(lnw[:], 1.0)
            nc.scalar.activation(lnw[:], lnw[:], Ln)

            p1T = wp.tile([P, 2, z1_tok], FP8, name="p1T", tag="p1T")
            p2T = wp.tile([PROJ2, z2_tok], BF16, name="p2T", tag="p2T")
            sep_h = wp.tile([P, TBC * NCH], F32, name="sep_h", tag="seph")
            sep_1 = wp.tile([P, t1b * NC1], F32, name="sep_1", tag="sep1")
            sep_2 = wp.tile([P, t2b * NC2], F32, name="sep_2", tag="sep2")
            ll_loc = wp.tile([P, TBC], F32, name="ll_loc", tag="llloc")
            if with_bias:
                ones_bf = wp.tile([1, P], BF16, name="ones_bf", tag="onesb")
                nc.gpsimd.memset(ones_bf[:], 1.0)

            def z_chunks(zt, w, mm_emit, bias_tile=None):
                """Fill zt[:, 0:w] by 512-col chunks via mm_emit(sl, first)."""
                for c0 in range(0, w, 512):
                    cw = min(512, w - c0)
                    sl = slice(c0, c0 + cw)
                    first = True
                    if bias_tile is not None:
                        nc.tensor.matmul(zt[0:P, sl], ones_bf[:],
                                         bias_tile[:, sl],
                                         start=True, stop=False)
                        first = False
                    mm_emit(sl, first)

            # exp goes PSUM -> SBUF bf16 (frees the PSUM tile at the ACT,
            # keeping the PE fed); row-sums run on VectorE, fusing PAIRS of
            # full-width exp tiles into one scalar_tensor_tensor with
            # accum_out (both SBUF read ports active -> half the reduce
            # time; accum = sum(A*1 + B))
            pending = {}
            next_col = {}
            dump = wp.tile([P, SS], BF16, name="dump", tag="dump")

            def exp_red(key, s, pair_limit, zt, w, sep, ncols, scale):
                et = scr.tile([P, SS], BF16, name="et", tag="et")
                nc.scalar.activation(et[:, 0:w], zt[:, 0:w], Exp,
                                     scale=scale)
                col = next_col.get(key, 0)
                base = key[1] * ncols
                if s < pair_limit:
                    if key in pending:
                        pt = pending.pop(key)
                        nc.vector.scalar_tensor_tensor(
                            out=dump[:, 0:w], in0=pt[:, 0:w], scalar=1.0,
                            in1=et[:, 0:w], op0=MUL, op1=ADD,
                            accum_out=sep[:, base + col:base + col + 1])
                        next_col[key] = col + 1
                    else:
                        pending[key] = et
                else:
                    nc.vector.tensor_reduce(
                        out=sep[:, base + col:base + col + 1],
                        in_=et[:, 0:w], axis=AX, op=ADD)
                    next_col[key] = col + 1

            # ---- main rounds: per superstrip r, head (4 blocks) + tail
            # tiles, interleaved so cheap tail matmuls sit between the
            # expensive head ones and ScalarE never builds a burst backlog
            for r in range(NSH):
                if r == 0:
                    hwt = hwt0
                else:
                    hwt = hwp.tile([P, HK, SS], FP8, name="hwt", tag="hwt")
                    nc.sync.dma_start(hwt[:, 0:HK // 2, :],
                                      hw_d[r, :, 0:HK // 2, :])
                    nc.gpsimd.dma_start(hwt[:, HK // 2:HK, :],
                                        hw_d[r, :, HK // 2:HK, :])
                if with_bias:
                    hbt = hwp.tile([1, SS], BF16, name="hbt", tag="hbt")
                    nc.sync.dma_start(hbt[:], hb_d[r])

                tail_jobs = []
                if r < NS1:
                    o1t = o1p.tile([P, 2, SS], FP8, name="o1t", tag="o1t")
                    nc.sync.dma_start(o1t[:], ow1_d[r])
                    o1bt = None
                    if with_bias:
                        o1bt = o1p.tile([1, SS], BF16, name="o1bt",
                                        tag="o1bt")
                        nc.sync.dma_start(o1bt[:], ob1_d[r])
                    w1 = T1W[r]

                    def t1_job(b, o1t=o1t, o1bt=o1bt, w1=w1, r=r):
                        zt = zs.tile([P, SS], F32, name="zt1", tag="zs")

                        def t1_mm(sl, first):
                            nc.tensor.matmul(
                                zt[0:P, sl],
                                p1T[:, :, b * P:(b + 1) * P],
                                o1t[:, :, sl],
                                start=first, stop=True, perf_mode=DR)

                        z_chunks(zt, w1, t1_mm, o1bt)
                        exp_red(("1", b), r, PL1, zt, w1, sep_1, NC1,
                                1.0 / W_SCALE)

                    for b in range(t1b):
                        tail_jobs.append((t1_job, b))
                if r % 2 == 0 and r // 2 < NS2:
                    s2 = r // 2
                    o2t = o2p.tile([PROJ2, SS], BF16, name="o2t", tag="o2t")
                    nc.gpsimd.dma_start(o2t[:], ow2_d[s2])
                    o2bt = None
                    if with_bias:
                        o2bt = o2p.tile([1, SS], BF16, name="o2bt",
                                        tag="o2bt")
                        nc.gpsimd.dma_start(o2bt[:], ob2_d[s2])
                    w2 = T2W[s2]

                    def t2_job(b, o2t=o2t, o2bt=o2bt, w2=w2, s2=s2):
                        zt = zs.tile([P, SS], F32, name="zt2", tag="zs")

                        def t2_mm(sl, first):
                            nc.tensor.matmul(
                                zt[0:P, sl],
                                p2T[:, b * P:(b + 1) * P],
                                o2t[:, sl],
                                start=first, stop=True)

                        z_chunks(zt, w2, t2_mm, o2bt)
                        exp_red(("2", b), s2, PL2, zt, w2, sep_2, NC2, 1.0)

                    for b in range(t2b):
                        tail_jobs.append((t2_job, b))

                w = HVW[r]
                for b in range(TBC):
                    zt = zs.tile([P, SS], F32, name="zt", tag="zs")

                    # k-major emission: all 512-chunks at k2 before moving to
                    # the next k2, so the PE streams each K-half of the
                    # weight strip as soon as its DMA lands
                    chunks = []
                    for c0 in range(0, w, 512):
                        cw = min(512, w - c0)
                        sl = slice(c0, c0 + cw)
                        if with_bias:
                            nc.tensor.matmul(zt[0:P, sl], ones_bf[:],
                                             hbt[:, sl],
                                             start=True, stop=False)
                        chunks.append(sl)
                    for k2 in range(K2N):
                        for sl in chunks:
                            nc.tensor.matmul(
                                zt[0:P, sl],
                                xt[:, b, 2 * k2:2 * k2 + 2, :],
                                hwt[:, 2 * k2:2 * k2 + 2, sl],
                                start=(k2 == 0 and not with_bias),
                                stop=(k2 == K2N - 1), perf_mode=DR)
                    exp_red(("h", b), r, PLH, zt, w, sep_h, NCH,
                            1.0 / W_SCALE)
                    if r > 0 and b < len(tail_jobs):
                        job, jb = tail_jobs[b]
                        job(jb)

                if r == 0:
                    # projections (tiny): tail proj activations for the
                    # routed zones, transposed into DoubleRow layouts
                    for m in range(PROJ1 // P):
                        acc = zs.tile([P, SS], F32, name="acc1", tag="zs")
                        for bb in range(t1b):
                            for k2 in range(K2N):
                                nc.tensor.matmul(
                                    acc[0:P, bb * P:(bb + 1) * P],
                                    pw1_t[:, 2 * k2:2 * k2 + 2,
                                          m * P:(m + 1) * P],
                                    xt[:, bb, 2 * k2:2 * k2 + 2, :],
                                    start=(k2 == 0), stop=(k2 == K2N - 1),
                                    perf_mode=DR)
                        nc.vector.tensor_copy(out=p1T[:, m, :],
                                              in_=acc[:, 0:z1_tok])
                    acc2 = zs.tile([P, SS], F32, name="acc2", tag="zs")
                    for bb in range(t2b):
                        for k2 in range(K2N):
                            nc.tensor.matmul(
                                acc2[0:PROJ2, bb * P:(bb + 1) * P],
                                pw2_t[:, 2 * k2:2 * k2 + 2, 0:PROJ2],
                                xt[:, t1b + bb, 2 * k2:2 * k2 + 2, :],
                                start=(k2 == 0), stop=(k2 == K2N - 1),
                                perf_mode=DR)
                    nc.vector.tensor_copy(out=p2T[:],
                                          in_=acc2[0:PROJ2, 0:z2_tok])
                    # round 0's tail tiles run after the projections
                    for job, jb in tail_jobs:
                        job(jb)

                # label-logit row-dots, one block per early round; DMAs on
                # the idle gpsimd queue so the sync FIFO never stalls the
                # weight stream
                if 3 <= r < 3 + 2 * TBC:
                    bl, half = divmod(r - 3, 2)
                    if half == 0:
                        xe = xw.tile([P, H], BF16, name="xe", tag="xe")
                        nc.sync.dma_start(xe[:], xb_d[bl])
                        xw_pend = xe
                    else:
                        we = xw.tile([P, H], BF16, name="we", tag="we")
                        nc.sync.dma_start(we[:], wb_d[bl])
                        lsc = xw.tile([P, H], BF16, name="lsc", tag="lsc")
                        nc.vector.scalar_tensor_tensor(
                            out=lsc[:], in0=xw_pend[:], scalar=1.0,
                            in1=we[:], op0=MUL, op1=MUL,
                            accum_out=ll_loc[:, bl:bl + 1])

                if r == 2:
                    padm_t = wp.tile([P, TBC], F32, name="padm_t", tag="padm")
                    nc.gpsimd.dma_start(padm_t[:], padm_d[:])
                    m1_t = wp.tile([P, TBC], F32, name="m1_t", tag="m1")
                    nc.gpsimd.dma_start(m1_t[:], m1_d[:])
                    m2_t = wp.tile([P, TBC], F32, name="m2_t", tag="m2")
                    nc.gpsimd.dma_start(m2_t[:], m2_d[:])
                    if with_bias:
                        llb_t = wp.tile([P, TBC], F32, name="llb_t",
                                        tag="llb")
                        nc.gpsimd.dma_start(llb_t[:], llb_d[:])

            # ---- final: per-token ln, mask, subtract label logit ----
            sev_h = sep_h.rearrange("p (b s) -> p b s", s=NCH)
            sh = wp.tile([P, TBC], F32, name="sh", tag="sh")
            nc.vector.tensor_reduce(out=sh[:], in_=sev_h, axis=AX, op=ADD)
            nc.vector.tensor_scalar_add(sh[:], sh[:], -float(N_PAD_HEAD))
            ln_h = wp.tile([P, TBC], F32, name="ln_h", tag="lnh")
            nc.scalar.activation(ln_h[:], sh[:], Ln)

            sev_1 = sep_1.rearrange("p (b s) -> p b s", s=NC1)
            s1 = wp.tile([P, t1b], F32, name="s1", tag="s1")
            nc.vector.tensor_reduce(out=s1[:], in_=sev_1, axis=AX, op=ADD)
            ln_1 = wp.tile([P, t1b], F32, name="ln_1", tag="ln1")
            nc.scalar.activation(ln_1[:], s1[:], Ln)

            sev_2 = sep_2.rearrange("p (b s) -> p b s", s=NC2)
            s2t = wp.tile([P, t2b], F32, name="s2t", tag="s2t")
            nc.vector.tensor_reduce(out=s2t[:], in_=sev_2, axis=AX, op=ADD)
            ln_2 = wp.tile([P, t2b], F32, name="ln_2", tag="ln2")
            nc.scalar.activation(ln_2[:], s2t[:], Ln)

            acc_l = wp.tile([P, TBC], F32, name="acc_l", tag="accl")
            tmp_l = wp.tile([P, TBC], F32, name="tmp_l", tag="tmpl")
            nc.vector.tensor_mul(out=acc_l[:], in0=padm_t[:], in1=ln_h[:])
            nc.vector.tensor_mul(out=tmp_l[:, 0:t1b], in0=m1_t[:, 0:t1b],
                                 in1=ln_1[:])
            nc.vector.tensor_add(out=acc_l[:, 0:t1b], in0=acc_l[:, 0:t1b],
                                 in1=tmp_l[:, 0:t1b])
            nc.vector.tensor_mul(out=tmp_l[:, 0:t2b],
                                 in0=m2_t[:, t1b:t1b + t2b], in1=ln_2[:])
            nc.vector.tensor_add(out=acc_l[:, t1b:t1b + t2b],
                                 in0=acc_l[:, t1b:t1b + t2b],
                                 in1=tmp_l[:, 0:t2b])
            nc.vector.tensor_sub(out=acc_l[:], in0=acc_l[:], in1=ll_loc[:])
            if with_bias:
                nc.vector.tensor_sub(out=acc_l[:], in0=acc_l[:],
                                     in1=llb_t[:])

            lred = wp.tile([P, 1], F32, name="lred", tag="lred")
            nc.vector.tensor_reduce(out=lred[:], in_=acc_l[:],
                                    axis=AX, op=ADD)
            nc.sync.dma_start(out_d[:], lred[:])

    nc.compile()
    return nc


def _get_nc(cfg):
    if cfg not in _NC_CACHE:
        _NC_CACHE[cfg] = _build_graph(cfg)
    return _NC_CACHE[cfg]


def kernel(inp, labels, head_w, head_b, t1_pw, t1_pb, t1_ow, t1_ob,
           t2_pw, t2_pb, t2_ow, t2_ob):
    global LAST_EXEC_NS, LAST_TRACE
    from concourse.bass_utils import run_bass_kernel_spmd

    inp = np.asarray(inp, dtype=np.float32)
    labels = np.asarray(labels)
    head_w = np.asarray(head_w, dtype=np.float32)
    head_b = np.asarray(head_b, dtype=np.float32)
    t1_pw = np.asarray(t1_pw, dtype=np.float32)
    t1_pb = np.asarray(t1_pb, dtype=np.float32)
    t1_ow = np.asarray(t1_ow, dtype=np.float32)
    t1_ob = np.asarray(t1_ob, dtype=np.float32)
    t2_pw = np.asarray(t2_pw, dtype=np.float32)
    t2_pb = np.asarray(t2_pb, dtype=np.float32)
    t2_ow = np.asarray(t2_ow, dtype=np.float32)
    t2_ob = np.asarray(t2_ob, dtype=np.float32)

    x0 = np.ascontiguousarray(inp.reshape(N, H))
    lab0 = labels.reshape(N).astype(np.int64)

    # token permutation: per core, tail1 zone first, then tail2 zone,
    # head-only fill
    m1_0 = (lab0 >= CUT0) & (lab0 < CUT1)
    m2_0 = lab0 >= CUT1
    idx1 = np.where(m1_0)[0]
    idx2 = np.where(m2_0)[0]
    idx0 = np.where(~(m1_0 | m2_0))[0]
    n1, n2 = len(idx1), len(idx2)
    t1b = max(1, -(-n1 // (N_CORES * P)))   # per-core tail1 blocks
    t2b = max(1, -(-n2 // (N_CORES * P)))
    if t1b + t2b > TBC:
        raise NotImplementedError(
            "label distribution exceeds routed-zone capacity")
    chunks1 = np.array_split(idx1, N_CORES)
    chunks2 = np.array_split(idx2, N_CORES)
    fill = list(idx0)
    fp = 0
    perms = []
    for c in range(N_CORES):
        parts = [chunks1[c]]
        need = t1b * P - len(chunks1[c])
        parts.append(np.asarray(fill[fp:fp + need], dtype=np.int64))
        fp += need
        parts.append(chunks2[c])
        need = t2b * P - len(chunks2[c])
        parts.append(np.asarray(fill[fp:fp + need], dtype=np.int64))
        fp += need
        need = TOK - t1b * P - t2b * P
        parts.append(np.asarray(fill[fp:fp + need], dtype=np.int64))
        fp += need
        perms.append(np.concatenate(parts))
    assert fp == len(fill)
    perm = np.concatenate(perms)
    assert perm.size == N

    x = x0[perm]
    lab = lab0[perm]

    m1 = (lab >= CUT0) & (lab < CUT1)
    m2 = lab >= CUT1
    pad = (lab != 0).astype(np.float32)
    head_labels = np.where(m1, CUT0, np.where(m2, CUT0 + 1, lab))
    lab1 = np.clip(lab - CUT0, 0, V1 - 1)
    lab2 = np.clip(lab - CUT1, 0, V2 - 1)
    m1f = m1.astype(np.float32)
    m2f = m2.astype(np.float32)

    with_bias = any(float(np.abs(b).max()) != 0.0
                    for b in (head_b, t1_pb, t1_ob, t2_pb, t2_ob))

    # effective label-weight columns, tails folded through their projections
    wl = head_w[:, head_labels]                      # [H, N]
    wl1 = t1_pw @ t1_ow[:, lab1]                     # [H, N]
    wl2 = t2_pw @ t2_ow[:, lab2]                     # [H, N]
    WLAB = (wl + m1f[None, :] * wl1 + m2f[None, :] * wl2) * pad[None, :]
    wlab_nat = np.ascontiguousarray(WLAB.T).astype(BF16_NP)      # [N, H]
    x_bf = x.astype(BF16_NP)

    import concourse.mybir as _mybir
    FP8_NP = _mybir.dt.np(_mybir.dt.float8e4)

    def pack_pairs(Xt):
        # [K, F] -> [128, K//128, F] with [p, kk, f] = Xt[kk*128 + p, f]
        K_, F_ = Xt.shape
        return np.ascontiguousarray(
            Xt.reshape(K_ // P, P, F_).transpose(1, 0, 2))

    def pack_strips(Xt, nss):
        # [K, nss*SS] -> [nss, 128, K//128, SS] strip-major pair layout
        K_ = Xt.shape[0]
        return np.ascontiguousarray(
            Xt.reshape(K_ // P, P, nss, SS).transpose(2, 1, 0, 3))

    hw_s = np.zeros((H, NSH * SS), dtype=np.float32)
    hw_s[:, :HEAD_DIM] = head_w * W_SCALE
    hw_pack = pack_strips(hw_s, NSH).astype(FP8_NP)

    o1_s = np.zeros((PROJ1, NS1 * SS), dtype=np.float32)
    o1_s[:, :V1] = t1_ow
    ow1_pack = pack_strips(o1_s, NS1).astype(FP8_NP)

    o2_s = np.zeros((PROJ2, NS2 * SS), dtype=np.float32)
    o2_s[:, :V2] = t2_ow * (1.0 / W_SCALE)
    ow2_pack = np.ascontiguousarray(
        o2_s.reshape(PROJ2, NS2, SS).transpose(1, 0, 2)).astype(BF16_NP)

    pw1_f8 = pack_pairs(t1_pw * W_SCALE).astype(FP8_NP)
    pw2_f8 = pack_pairs(t2_pw * W_SCALE).astype(FP8_NP)

    if with_bias:
        hb_s = np.zeros((NSH * SS,), dtype=np.float32)
        hb_s[:HEAD_DIM] = head_b * W_SCALE
        hb_pack = hb_s.reshape(NSH, 1, SS).astype(BF16_NP)
        ob1_s = np.zeros((NS1 * SS,), dtype=np.float32)
        ob1_s[:V1] = (t1_ob + t1_pb @ t1_ow) * W_SCALE
        ob1_pack = ob1_s.reshape(NS1, 1, SS).astype(BF16_NP)
        ob2_s = np.zeros((NS2 * SS,), dtype=np.float32)
        ob2_s[:V2] = t2_ob + t2_pb @ t2_ow
        ob2_pack = ob2_s.reshape(NS2, 1, SS).astype(BF16_NP)
        llb_vec = pad * (head_b[head_labels]
                         + m1f * (t1_pb @ t1_ow[:, lab1] + t1_ob[lab1])
                         + m2f * (t2_pb @ t2_ow[:, lab2] + t2_ob[lab2]))

    def to_ptb(v):
        return np.ascontiguousarray(
            v.reshape(TBC, P).T).astype(np.float32)   # [P, TBC]

    in_maps = []
    for c in range(N_CORES):
        tsl = slice(c * TOK, (c + 1) * TOK)
        xc = x[tsl]
        xt_blocks = np.stack([
            pack_pairs(np.ascontiguousarray(xc[b * P:(b + 1) * P].T))
            for b in range(TBC)])                    # [TBC, P, HK, 128]
        m = {
            "xT": xt_blocks.astype(FP8_NP),
            "xb": np.ascontiguousarray(x_bf[tsl].reshape(TBC, P, H)),
            "wb": np.ascontiguousarray(wlab_nat[tsl].reshape(TBC, P, H)),
            "hw": hw_pack,
            "ow1": ow1_pack,
            "ow2": ow2_pack,
            "pw1": pw1_f8,
            "pw2": pw2_f8,
            "padm": to_ptb(pad[tsl]),
            "m1m": to_ptb(m1f[tsl]),
            "m2m": to_ptb(m2f[tsl]),
        }
        if with_bias:
            m["hb"] = hb_pack
            m["ob1"] = ob1_pack
            m["ob2"] = ob2_pack
            m["llb"] = to_ptb(llb_vec[tsl])
        in_maps.append(m)

    nc = _get_nc((t1b, t2b, with_bias))
    trace = bool(os.environ.get("KERNEL_TRACE"))
    if trace:
        _ensure_trace_hook()
    # the fleet occasionally throws transient NRT device errors on the first
    # execution after a crashed run; retry a couple of times
    res = None
    for attempt in range(3):
        try:
            res = run_bass_kernel_spmd(
                nc, in_maps, core_ids=list(range(N_CORES)), trace=trace)
            break
        except Exception:
            if attempt == 2:
                raise
            import time
            time.sleep(3.0)
    LAST_EXEC_NS = res.exec_time_ns
    LAST_TRACE = res.instructions_and_trace
    total = 0.0
    for c in range(N_CORES):
        total += float(res.results[c]["out"].astype(np.float64).sum())
    return np.float32(total / N)
